# revision 18
# baseline (speedup 1.0000x reference)
"""AttentionMM kernel for Trainium2 (Bass/Tile), data-parallel over 8 NeuronCores.

Math (per batch b, with x1,x2: (T,E)):
    S = x1 @ x2^T  is never materialized:
        t1 = sum_i x1[i,:] ;  t2 = sum_j x2[j,:]
        G2 = x1^T @ x2  (E,E)
        c1 = (1/T) G2^T t2 ;  c2 = (1/T) G2 t1   (via G = x2^T x1 = G2^T)
    et1 = c1 @ U1 + x1 @ W1 + b1 ;  et2 = c2 @ U2 + x2 @ W2 + b2
    o1 = softmax(et1) @ x1 ;  o2 = softmax(et2) @ x2 ;  out = [o1 | o2]

Implementation notes:
  - Everything on-chip is float16: fp16 matmuls run at 1 cycle/row on the PE
    (vs 4 cycles/row for f32r below 256 moving columns) and halve HBM
    traffic.  PSUM accumulation stays fp32, so the only precision loss is
    input rounding (2^-11) - measured end-to-end rel err ~7e-3.
  - Tokens sit in SBUF partitions, p-major: token t = p*16 + k, so each
    DMA moves 16 consecutive 260B rows per partition (4KB+ contiguous
    chunks on both sides -> good HBM descriptor efficiency).
  - The host appends a ones-column to x (E -> 129 cols, padded to 130):
    Gram matmuls then yield the token-sums t1/t2 for free, and readout
    matmuls yield the softmax denominator Z for free.
  - Softmax uses a constant shift (-27) instead of a max-subtraction.
    The shift cancels exactly in o = (sum ex*x)/Z; -27 keeps every
    batch's exp() inside fp16 range (global max logit ~37 -> e^10~22K
    < 65504; weakest batch max logit ~15 -> e^-11.6 ~ 9e-6, still ~150
    denormal steps).
  - Both attention sides are packed side by side ([x1|1|0|x2|1|0] per
    token) so per-batch elementwise work is one DVE op per stage and the
    readout runs as a single M=2, N=260 accumulating matmul per k-tile.
  - x@W runs on DVE (fused multiply then X-axis reduce, fp16 2x/4x
    modes); GPSIMD only does small copies/adds (its big-op throughput is
    poor).
"""

import numpy as np

import concourse.bass as bass
import concourse.mybir as mybir
import concourse.tile as tile
from concourse.bass_utils import run_bass_kernel_spmd

B, T, E = 32, 2048, 128
NCORES = 8
BPC = B // NCORES            # batches per core
KT = T // 128                # token tiles per batch
CW = E + 2                   # row width: 128 x-cols + ones col + pad
F32 = mybir.dt.float32
F16 = mybir.dt.float16
AF = mybir.ActivationFunctionType
ALU = mybir.AluOpType
INV_T = 1.0 / T


def _patch_sem_clear():
    """The installed walrus cannot encode EVENT_SEMAPHORE_RANGE_CLEAR (raw
    ISA, "ISA wrong length"), which TileContext's exit path emits via
    gpsimd.sem_clear. Skip the clear (keep the DMA drain + bookkeeping);
    the runtime re-initializes semaphore state per NEFF execution."""
    if getattr(bass.Bass, "_semclear_patched", False):
        return
    from concourse.bass import compact_to_ranges

    def patched(self, sems):
        if not sems:
            return
        sem_nums = [s.num if hasattr(s, "num") else s for s in sems]
        for sem_range in compact_to_ranges(sem_nums):
            assert self._state.free_isdisjoint(sem_range)
            self.gpsimd.dma_reset(sem_range)
        self._state.prepend_free_semaphores(sem_nums)
        for poison_set in self._tile_sem_poison_stack:
            poison_set.update(sem_nums)

    bass.Bass.clear_and_free_semaphores = patched
    bass.Bass._semclear_patched = True


def _legalize_sync_waits(nc):
    """The installed walrus encodes at most one sync-wait per instruction
    ("Too many sync wait commands"). Move excess waits onto engine NoOps
    inserted immediately before the instruction — same engine, same
    program position, so semantics are unchanged."""
    import bass_rust

    fn = nc.m.functions[0]
    n_nops = 0
    for blk in fn.blocks:
        insts = blk.instructions
        out = []
        dirty = False
        for inst in insts:
            si = inst.sync_info
            if si is not None and len(si.on_wait) > 1:
                waits = list(si.on_wait)
                for w in waits[:-1]:
                    nop = mybir.InstNoOp(
                        name=f"waitnop-{n_nops}", engine=inst.engine
                    )
                    nop.sync_info = bass_rust.SyncInfo(
                        on_wait=[w], on_update=[]
                    )
                    out.append(nop)
                    n_nops += 1
                inst.sync_info = bass_rust.SyncInfo(
                    on_wait=[waits[-1]], on_update=list(si.on_update)
                )
                dirty = True
            out.append(inst)
        if dirty:
            blk.instructions = out
    return n_nops


def _build():
    _patch_sem_clear()
    nc = bass.Bass(
        "TRN2", target_bir_lowering=False, debug=False, num_devices=NCORES
    )

    # x: both sides packed, p-major tokens: row (b, s, p, k) = x_s[b, p*16+k]
    xd = nc.dram_tensor(
        "xc", (BPC * 2, 128, KT, CW), F16, kind="ExternalInput"
    ).ap()
    ud = nc.dram_tensor("u12", (2, E, T), F16, kind="ExternalInput").ap()
    wd = nc.dram_tensor("w12bc", (128, 2, E), F16, kind="ExternalInput").ap()
    bd = nc.dram_tensor("b12s", (128, 2, KT), F16, kind="ExternalInput").ap()
    outd = nc.dram_tensor("out", (2, BPC * E), F32, kind="ExternalOutput").ap()

    with tile.TileContext(nc) as tc:
        with (
            tc.tile_pool(name="const", bufs=1) as cpool,
            tc.tile_pool(name="xpool", bufs=1) as xpool,
            tc.tile_pool(name="work", bufs=2) as wpool,
            tc.tile_pool(name="ps", bufs=1, space="PSUM") as pspool,
        ):
            # ---- persistent tiles ----
            U12s = cpool.tile([128, 2, T], F16, tag="u12")
            W12 = cpool.tile([128, 2, E], F16, tag="w12")
            B12 = cpool.tile([128, 2, KT], F16, tag="b12")
            C12 = cpool.tile([128, 2, BPC], F16, tag="c12")
            XWB = cpool.tile([128, BPC, 2, KT], F16, tag="xwb")
            OUT = cpool.tile([128, BPC * E], F32, tag="outbuf")

            # Params + U on the ACT ring, issued first (~2.7us), so ACT is
            # free for compute afterwards; ALL x traffic on the sync ring,
            # sides interleaved so each batch completes as early as possible.
            nc.scalar.dma_start(W12[:], wd)
            nc.scalar.dma_start(B12[:], bd)
            nc.scalar.dma_start(U12s[:, 0, :], ud[0])
            nc.scalar.dma_start(U12s[:, 1, :], ud[1])

            XB = []
            for b in range(BPC):
                xt = xpool.tile([128, 2, KT, CW], F16, tag=f"x_{b}")
                # batch 0 in quarters (earlier compute start), rest in halves
                nh = 4 if b == 0 else 2
                kq = KT // nh
                for h in range(nh):
                    ks = slice(h * kq, (h + 1) * kq)
                    nc.sync.dma_start(xt[:, 0, ks, :], xd[2 * b][:, ks])
                    nc.sync.dma_start(xt[:, 1, ks, :], xd[2 * b + 1][:, ks])
                XB.append(xt)

                # x@W: issued with the loads so DVE starts early.  Multiply
                # at fp16 2x rate, then a tensor_tensor fold tree (DVE's
                # plain reduce runs at 1x; the tree stays at 2x throughout).
                scr = wpool.tile([128, 2, KT, E], F16, tag="scr", bufs=2)
                nc.vector.tensor_tensor(
                    scr[:],
                    xt[:, :, :, 0:E],
                    W12.unsqueeze(2).broadcast_to((128, 2, KT, E)),
                    ALU.mult,
                )
                src = scr
                w = E
                while w > 1:
                    h = w // 2
                    dst = wpool.tile([128, 2, KT, h], F16, tag=f"xf{h}", bufs=2)
                    nc.vector.tensor_tensor(
                        dst[:], src[:, :, :, 0:h], src[:, :, :, h:w], ALU.add
                    )
                    src = dst
                    w = h
                nc.vector.tensor_add(XWB[:, b], src[:, :, :, 0], B12[:])

            psE = pspool.tile([128, 2, KT, BPC], F32, tag="psE", bufs=1)

            # ---- per-batch: Gram phases, c's ----
            for b in range(BPC):
                xt = XB[b]

                # phase A: [G2 | t1] = x1^T @ [x2 | 1]
                psA = pspool.tile([128, CW], F32, tag="psA", bufs=2)
                for k in range(KT):
                    nc.tensor.matmul(
                        psA[:],
                        xt[:, 0, k, 0:E],
                        xt[:, 1, k, :],
                        start=(k == 0),
                        stop=(k == KT - 1),
                    )
                # phase B: [G | t2] = x2^T @ [x1 | 1]
                psB = pspool.tile([128, CW], F32, tag="psB", bufs=2)
                for k in range(KT):
                    nc.tensor.matmul(
                        psB[:],
                        xt[:, 1, k, 0:E],
                        xt[:, 0, k, :],
                        start=(k == 0),
                        stop=(k == KT - 1),
                    )

                GA = wpool.tile([128, CW], F16, tag="ga", bufs=2)
                GB = wpool.tile([128, CW], F16, tag="gb", bufs=2)
                nc.scalar.copy(GA[:], psA[:])
                nc.scalar.copy(GB[:], psB[:])

                # TC = [t1 | t2]; then:
                #   lhsT=G2 (=GA), rhs=TC -> col1: G2^T t2 = T*c1
                #   lhsT=G  (=GB), rhs=TC -> col0: G^T t1  = T*c2
                TC = wpool.tile([128, 2], F16, tag="tc", bufs=2)
                nc.vector.tensor_copy(TC[:, 0:1], GA[:, E : E + 1])
                nc.vector.tensor_copy(TC[:, 1:2], GB[:, E : E + 1])
                psC = pspool.tile([128, 4], F32, tag="psC", bufs=1)
                nc.tensor.matmul(psC[:, 0:2], GA[:, 0:E], TC[:], start=True, stop=True)
                nc.tensor.matmul(psC[:, 2:4], GB[:, 0:E], TC[:], start=True, stop=True)
                # scale by 1/T while casting into the batched c matrix
                nc.scalar.mul(C12[:, 0, b : b + 1], psC[:, 1:2], INV_T)
                nc.scalar.mul(C12[:, 1, b : b + 1], psC[:, 2:3], INV_T)

            # ---- U phase: et contributions for all batches at once ----
            # token t = p*16+k  ->  U column for (p, k) is U[:, p*16+k];
            # the host pre-permutes U so tile k's columns are contiguous.
            for s in range(2):
                for k in range(KT):
                    nc.tensor.matmul(
                        psE[:, s, k, :],
                        U12s[:, s, k * 128 : (k + 1) * 128],
                        C12[:, s, :],
                        start=True,
                        stop=True,
                    )

            # ---- per-batch: logits + per-partition maxima ----
            ETs = []
            mxall = cpool.tile([128, 2 * BPC], F32, tag="mxall")
            for b in range(BPC):
                et = wpool.tile([128, 2, KT], F32, tag="et", bufs=BPC)
                nc.vector.scalar_tensor_tensor(
                    out=et[:],
                    in0=psE[:, :, :, b],
                    scalar=1.0,
                    in1=XWB[:, b],
                    op0=ALU.mult,
                    op1=ALU.add,
                )
                # col j = 2*b + s
                nc.vector.tensor_reduce(
                    out=mxall[:, 2 * b : 2 * b + 2], in_=et[:],
                    axis=mybir.AxisListType.X, op=ALU.max,
                )
                ETs.append(et)

            # ---- cross-partition max (true per-side max subtraction keeps
            # exp() inside fp16 range for any data and makes softmax exact).
            # The installed walrus can't encode gpsimd partition reduces, so
            # fold on DVE: quadrant shuffles + 32x32 block transpose. ----
            idmask = list(range(32))
            macc = wpool.tile([32, 2 * BPC], F32, tag="macc")
            mtmp = wpool.tile([32, 2 * BPC], F32, tag="mtmp")
            nc.vector.tensor_copy(macc[:], mxall[0:32, :])
            for q in (1, 2, 3):
                nc.vector.stream_shuffle(
                    mtmp[:], mxall[32 * q : 32 * q + 32, :], mask=idmask
                )
                nc.vector.tensor_tensor(macc[:], macc[:], mtmp[:], ALU.max)
            p32 = wpool.tile([32, 32], F32, tag="mp32")
            nc.vector.memset(p32[:], -3.0e38)
            nc.vector.tensor_copy(p32[:, 0 : 2 * BPC], macc[:])
            t32 = wpool.tile([32, 32], F32, tag="mt32")
            nc.vector.transpose(t32[:], p32[:])
            nm = wpool.tile([32, 1], F32, tag="mnm")
            nc.vector.tensor_reduce(
                out=nm[:], in_=t32[:], axis=mybir.AxisListType.X, op=ALU.max
            )
            nmneg = wpool.tile([32, 1], F32, tag="mneg")
            nc.vector.tensor_scalar_mul(nmneg[:], nm[:], -1.0)
            # row-ify ([32,1] -> row 0 of [32,32]) then broadcast to all
            # partitions: shuffle mask 0 (every lane reads lane 0), then
            # quadrant copies.
            q32 = wpool.tile([32, 32], F32, tag="mq32")
            nc.vector.memset(q32[:], 0.0)
            nc.vector.tensor_copy(q32[:, 0:1], nmneg[:])
            r32 = wpool.tile([32, 32], F32, tag="mr32")
            nc.vector.transpose(r32[:], q32[:])
            nbias = cpool.tile([128, 32], F32, tag="nbias")
            nc.vector.stream_shuffle(nbias[0:32, :], r32[:], mask=[0] * 32)
            for q in (1, 2, 3):
                nc.vector.stream_shuffle(
                    nbias[32 * q : 32 * q + 32, :], nbias[0:32, :], mask=idmask
                )

            # ---- exp with per-(batch,side) bias ----
            EXs = []
            for b in range(BPC):
                EX = wpool.tile([128, 2, KT], F16, tag="ex", bufs=2)
                for s in range(2):
                    j = 2 * b + s
                    nc.scalar.activation(
                        EX[:, s, :], ETs[b][:, s, :], AF.Exp,
                        bias=nbias[:, j : j + 1],
                    )
                EXs.append(EX)

            # ---- readout in batch-pairs: 4 concurrent col-group streams ----
            # slot j = 2*s + bb (bb = b%2) -> PE col-group j, PSUM partition
            # 32*j.  Each slot accumulates EX_s^T [x_s | 1] over the 16
            # k-tiles; Z lands at col E via the ones column.
            for P in range(2):
                psO = pspool.tile([128, CW], F32, tag="psO", bufs=2)
                for k in range(KT):
                    for bb in range(2):
                        b = 2 * P + bb
                        for s in range(2):
                            j = 2 * s + bb
                            nc.tensor.matmul(
                                psO[32 * j : 32 * j + 1, :],
                                EXs[b][:, s, k : k + 1],
                                XB[b][:, s, k, :],
                                start=(k == 0),
                                stop=(k == KT - 1),
                                tile_position=(0, 32 * j),
                                skip_group_check=True,
                            )
                # normalize: out = o~ / Z
                rz = wpool.tile([128, 1], F32, tag="rz", bufs=2)
                for bb in range(2):
                    b = 2 * P + bb
                    for s in range(2):
                        j = 2 * s + bb
                        p0 = 32 * j
                        nc.vector.reciprocal(
                            rz[p0 : p0 + 1, :], psO[p0 : p0 + 1, E : E + 1]
                        )
                        nc.scalar.mul(
                            OUT[p0 : p0 + 1, b * E : (b + 1) * E],
                            psO[p0 : p0 + 1, 0:E],
                            rz[p0 : p0 + 1, :],
                        )

            # out rows: side s batches (bb, bb+2) live on partition 32*(2s+bb)
            for s in range(2):
                for bb in range(2):
                    p0 = 32 * (2 * s + bb)
                    src = OUT[p0 : p0 + 1, :].rearrange(
                        "p (P bb e) -> p bb P e", bb=2, e=E
                    )
                    dst = outd[s].rearrange("(P bb e) -> bb P e", bb=2, e=E)
                    nc.sync.dma_start(
                        dst[bb].unsqueeze(0), src[:, bb]
                    )

    return nc


_NC_CACHE = {}


def _get_nc():
    if "nc" not in _NC_CACHE:
        _NC_CACHE["nc"] = _build()
    return _NC_CACHE["nc"]


# U column permutation: tile k, lane j  <-  U[:, j*16 + k]
_UIDX = np.arange(T).reshape(128, KT).T.reshape(-1)


def _prep_in_maps(x1, x2, W1, b1, U1, W2, b2, U2):
    f16 = np.float16
    x1 = np.asarray(x1, dtype=np.float32)
    x2 = np.asarray(x2, dtype=np.float32)

    # packed x: (B, 2, 128, KT, CW) fp16, token t = p*16 + k, ones col at E
    xall = np.zeros((B, 2, 128, KT, CW), dtype=f16)
    xall[:, 0, :, :, 0:E] = x1.reshape(B, 128, KT, E).astype(f16)
    xall[:, 1, :, :, 0:E] = x2.reshape(B, 128, KT, E).astype(f16)
    xall[:, :, :, :, E] = 1.0

    u12 = np.stack(
        [
            np.asarray(U1, np.float32)[:, _UIDX].astype(f16),
            np.asarray(U2, np.float32)[:, _UIDX].astype(f16),
        ]
    )
    w12 = np.ascontiguousarray(
        np.broadcast_to(
            np.stack(
                [np.asarray(W1, f16)[:, 0], np.asarray(W2, f16)[:, 0]]
            )[None, :, :],
            (128, 2, E),
        )
    )
    b12 = np.ascontiguousarray(
        np.stack(
            [
                np.asarray(b1, f16)[:, 0].reshape(128, KT),
                np.asarray(b2, f16)[:, 0].reshape(128, KT),
            ],
            axis=1,
        )
    )

    in_maps = []
    for c in range(NCORES):
        sl = slice(c * BPC, (c + 1) * BPC)
        in_maps.append(
            {
                "xc": np.ascontiguousarray(xall[sl]).reshape(
                    BPC * 2, 128, KT, CW
                ),
                "u12": u12,
                "w12bc": w12,
                "b12s": b12,
            }
        )
    return in_maps


def _run(trace=False, tmpdir=None, **inputs):
    nc = _get_nc()
    if not _NC_CACHE.get("legalized"):
        # must happen after any CoreSim use (sim can't model bare wait-nops)
        _legalize_sync_waits(nc)
        _NC_CACHE["legalized"] = True
    in_maps = _prep_in_maps(**inputs)
    res = run_bass_kernel_spmd(
        nc, in_maps, list(range(NCORES)), trace=trace, tmpdir=tmpdir
    )
    # per-core out: (2, BPC*E) -> (BPC, 2E)
    outs = []
    for r in res.results:
        o = r["out"].reshape(2, BPC, E)
        outs.append(np.concatenate([o[0], o[1]], axis=1))
    out = np.concatenate(outs, axis=0)
    return out, res


def kernel(x1, x2, W1, b1, U1, W2, b2, U2):
    out, _ = _run(
        x1=x1, x2=x2, W1=W1, b1=b1, U1=U1, W2=W2, b2=b2, U2=U2
    )
    return out


# revision 41
# speedup vs baseline: 1.0551x; 1.0551x over previous
"""AttentionMM kernel for Trainium2 (Bass/Tile), data-parallel over 8 NeuronCores.

Math (per batch b, with x1,x2: (T,E)):
    S = x1 @ x2^T  is never materialized:
        t1 = sum_i x1[i,:] ;  t2 = sum_j x2[j,:]
        G2 = x1^T @ x2  (E,E)
        c1 = (1/T) G2^T t2 ;  c2 = (1/T) G2 t1   (via G = x2^T x1 = G2^T)
    et1 = c1 @ U1 + x1 @ W1 + b1 ;  et2 = c2 @ U2 + x2 @ W2 + b2
    o1 = softmax(et1) @ x1 ;  o2 = softmax(et2) @ x2 ;  out = [o1 | o2]

Implementation notes:
  - Everything on-chip is float16: fp16 matmuls run at 1 cycle/row on the PE
    (vs 4 cycles/row for f32r below 256 moving columns) and halve HBM
    traffic.  PSUM accumulation stays fp32; measured rel err ~4e-3.
  - Tokens sit in SBUF partitions, p-major: token t = p*16 + k.  Batches are
    packed in PAIRS ([128, 2, 2, KT, CW] tiles) so elementwise stages run as
    one DVE op per pair.
  - The host appends a ones-column to x: Gram matmuls yield t1/t2 for free
    and readout matmuls yield the softmax denominator Z for free.
  - Softmax subtracts a true per-(batch,side) max so exp() stays in fp16
    range for any input scale.  The cross-partition max/broadcast runs on
    the PE (transpose-mode with a host-supplied fp16 identity + a K=1
    ones-matmul broadcast) - the walrus here can't encode gpsimd partition
    reduces, and DVE stream-shuffle tables inflate the NEFF preamble.
  - x@W runs on DVE: fp16 multiply at 2x rate, then a tensor_tensor fold
    tree (DVE's plain reduce runs at 1x, tensor_tensor at 2x).
  - Readout packs (side, batch%2) into the PE's four 32-wide column groups
    via tile_position, so four N=130 streams run concurrently and each
    (batch, side) readout row lands on its own partition (0/32/64/96),
    which also keeps the per-side normalize ops on legal partition bases.
"""

import numpy as np

import concourse.bass as bass
import concourse.mybir as mybir
import concourse.tile as tile
from concourse.bass_utils import run_bass_kernel_spmd

B, T, E = 32, 2048, 128
NCORES = 8
BPC = B // NCORES            # batches per core
NP = BPC // 2                # batch pairs per core
KT = T // 128                # token tiles per batch
CW = E + 2                   # row width: 128 x-cols + ones col + pad
F32 = mybir.dt.float32
F16 = mybir.dt.float16
AF = mybir.ActivationFunctionType
ALU = mybir.AluOpType
INV_T = 1.0 / T
DEBUG_TAPS = False


def _patch_sem_clear():
    """The installed walrus cannot encode EVENT_SEMAPHORE_RANGE_CLEAR (raw
    ISA, "ISA wrong length"), which TileContext's exit path emits via
    gpsimd.sem_clear. Skip the clear (keep the DMA drain + bookkeeping);
    the runtime re-initializes semaphore state per NEFF execution."""
    if getattr(bass.Bass, "_semclear_patched", False):
        return
    from concourse.bass import compact_to_ranges

    def patched(self, sems):
        if not sems:
            return
        sem_nums = [s.num if hasattr(s, "num") else s for s in sems]
        for sem_range in compact_to_ranges(sem_nums):
            assert self._state.free_isdisjoint(sem_range)
            self.gpsimd.dma_reset(sem_range)
        self._state.prepend_free_semaphores(sem_nums)
        for poison_set in self._tile_sem_poison_stack:
            poison_set.update(sem_nums)

    bass.Bass.clear_and_free_semaphores = patched
    bass.Bass._semclear_patched = True


def _legalize_sync_waits(nc):
    """The installed walrus encodes at most one sync-wait per instruction
    ("Too many sync wait commands"). Move excess waits onto engine NoOps
    inserted immediately before the instruction — same engine, same
    program position, so semantics are unchanged."""
    import bass_rust

    fn = nc.m.functions[0]
    n_nops = 0
    for blk in fn.blocks:
        insts = blk.instructions
        out = []
        dirty = False
        for inst in insts:
            si = inst.sync_info
            if si is not None and len(si.on_wait) > 1:
                waits = list(si.on_wait)
                for w in waits[:-1]:
                    nop = mybir.InstNoOp(
                        name=f"waitnop-{n_nops}", engine=inst.engine
                    )
                    nop.sync_info = bass_rust.SyncInfo(
                        on_wait=[w], on_update=[]
                    )
                    out.append(nop)
                    n_nops += 1
                inst.sync_info = bass_rust.SyncInfo(
                    on_wait=[waits[-1]], on_update=list(si.on_update)
                )
                dirty = True
            out.append(inst)
        if dirty:
            blk.instructions = out
    return n_nops


def _build():
    _patch_sem_clear()
    nc = bass.Bass(
        "TRN2", target_bir_lowering=False, debug=False, num_devices=NCORES
    )

    # x: (pair, bb, side) packed, p-major tokens: row = x_s[2P+bb, p*16+k]
    xd = nc.dram_tensor(
        "xc", (BPC * 2, 128, KT, CW), F16, kind="ExternalInput"
    ).ap()
    ud = nc.dram_tensor("u12", (2, E, T), F16, kind="ExternalInput").ap()
    wd = nc.dram_tensor("w12bc", (128, 4, E), F16, kind="ExternalInput").ap()
    bd = nc.dram_tensor("b12s", (128, 4, KT), F16, kind="ExternalInput").ap()
    idd = nc.dram_tensor("ident", (128, 128), F32, kind="ExternalInput").ap()
    outd = nc.dram_tensor("out", (2, BPC * E), F32, kind="ExternalOutput").ap()
    if DEBUG_TAPS:
        dbgd = nc.dram_tensor(
            "dbg", (128, 152), F32, kind="ExternalOutput"
        ).ap()
        gdbgd = nc.dram_tensor(
            "gdbg", (128, 2 * CW + 2), F32, kind="ExternalOutput"
        ).ap()

    with tile.TileContext(nc) as tc:
        with (
            tc.tile_pool(name="const", bufs=1) as cpool,
            tc.tile_pool(name="xpool", bufs=1) as xpool,
            tc.tile_pool(name="work", bufs=2) as wpool,
            tc.tile_pool(name="ps", bufs=1, space="PSUM") as pspool,
        ):
            # ---- persistent tiles ----
            U12s = cpool.tile([128, 2, T], F16, tag="u12")
            # W/b pre-expanded on host to 4 cols (j = 2*bb + s) so the
            # pair-merged DVE ops keep affine 3-free-dim APs
            W12 = cpool.tile([128, 4, E], F16, tag="w12")
            B12 = cpool.tile([128, 4, KT], F16, tag="b12")
            IDN = cpool.tile([128, 128], F32, tag="ident")
            ONES = cpool.tile([1, 128], F32, tag="ones")
            C12 = cpool.tile([128, 2, BPC], F16, tag="c12")
            XWB = cpool.tile([128, BPC, 2, KT], F16, tag="xwb")
            OUT = cpool.tile([128, BPC * E], F32, tag="outbuf")
            nc.vector.memset(ONES[:], 1.0)

            # Params + identity + U on the ACT ring first; ALL x1 on the
            # sync ring, x2 on the ACT ring behind the params.
            nc.scalar.dma_start(W12[:], wd)
            nc.scalar.dma_start(B12[:], bd)
            nc.scalar.dma_start(IDN[:], idd)

            XB = []
            for P in range(NP):
                xt = xpool.tile([128, 2, 2, KT, CW], F16, tag=f"x_{P}")
                XB.append(xt)
            KH = KT // 2
            for P in range(NP):
                xt = XB[P]
                for bb in range(2):
                    for h in range(2):
                        ks = slice(h * KH, (h + 1) * KH)
                        i = 2 * (2 * P + bb)
                        nc.sync.dma_start(xt[:, bb, 0, ks, :], xd[i][:, ks])
                        nc.scalar.dma_start(
                            xt[:, bb, 1, ks, :], xd[i + 1][:, ks]
                        )
                # x@W for the pair: one fp16 multiply + fold tree on DVE,
                # issued with the loads so DVE starts as data lands.
                scr = wpool.tile([128, 4, KT, E], F16, tag="scr", bufs=1)
                nc.vector.tensor_tensor(
                    scr[:],
                    xt[:, :, :, :, 0:E].rearrange("p b s k c -> p (b s) k c"),
                    W12.unsqueeze(2).broadcast_to((128, 4, KT, E)),
                    ALU.mult,
                )
                src = scr
                w = E
                while w > 1:
                    h = w // 2
                    dst = wpool.tile(
                        [128, 4, KT, h], F16, tag=f"xf{h}", bufs=1
                    )
                    nc.vector.tensor_tensor(
                        dst[:], src[:, :, :, 0:h], src[:, :, :, h:w],
                        ALU.add,
                    )
                    src = dst
                    w = h
                nc.vector.tensor_add(
                    XWB[:, 2 * P : 2 * P + 2].rearrange(
                        "p b s k -> p (b s) k"
                    ),
                    src[:, :, :, 0],
                    B12[:],
                )

            nc.sync.dma_start(U12s[:, 0, :], ud[0])
            nc.sync.dma_start(U12s[:, 1, :], ud[1])

            psE = pspool.tile([128, 2, KT, BPC], F32, tag="psE", bufs=1)
            # one shared bank for the small PSUM pieces of the max stage
            # (slices are disjoint; writes/reads are naturally ordered);
            # a second fp16-typed bank for PE-transpose outputs
            psX = pspool.tile([128, 512], F32, tag="psX", bufs=1)

            # ---- per-batch: Gram phases, c's ----
            GTAP = {}
            for P in range(NP):
                for bb in range(2):
                    b = 2 * P + bb
                    xt = XB[P]

                    # [G2 | t1] = x1^T @ [x2 | 1] ; [G | t2] = x2^T @ [x1 | 1]
                    # NOTE: start=True clears has_written BANK-wide, so the
                    # two accumulation groups sharing this bank must run
                    # sequentially (A fully, then B) — finished values
                    # persist, only the accumulate flags are cleared.
                    psAB = pspool.tile([128, 2, CW], F32, tag="psAB", bufs=2)
                    for k in range(KT):
                        nc.tensor.matmul(
                            psAB[:, 0, :],
                            xt[:, bb, 0, k, 0:E],
                            xt[:, bb, 1, k, :],
                            start=(k == 0),
                            stop=(k == KT - 1),
                        )
                    for k in range(KT):
                        nc.tensor.matmul(
                            psAB[:, 1, :],
                            xt[:, bb, 1, k, 0:E],
                            xt[:, bb, 0, k, :],
                            start=(k == 0),
                            stop=(k == KT - 1),
                        )

                    GAB = wpool.tile([128, 2, CW], F16, tag="gab", bufs=2)
                    nc.scalar.copy(GAB[:], psAB[:])

                    # TC = [t1 | t2]; then:
                    #   lhsT=G2, rhs=TC -> col1: G2^T t2 = T*c1
                    #   lhsT=G,  rhs=TC -> col0: G^T t1  = T*c2
                    TC = wpool.tile([128, 2], F16, tag="tc", bufs=2)
                    nc.vector.tensor_copy(TC[:], GAB[:, :, E])
                    psC = psX[:, 8 * b : 8 * b + 4]
                    nc.tensor.matmul(
                        psC[:, 0:2], GAB[:, 0, 0:E], TC[:],
                        start=True, stop=True,
                    )
                    nc.tensor.matmul(
                        psC[:, 2:4], GAB[:, 1, 0:E], TC[:],
                        start=True, stop=True,
                    )
                    # scale by 1/T while casting into the batched c matrix
                    nc.scalar.mul(C12[:, 0, b : b + 1], psC[:, 1:2], INV_T)
                    nc.scalar.mul(C12[:, 1, b : b + 1], psC[:, 2:3], INV_T)
                    if DEBUG_TAPS and b == 0:
                        gtap = cpool.tile([128, 2 * CW + 2], F32, tag="gtap")
                        nc.vector.tensor_copy(
                            gtap[:, 0 : 2 * CW],
                            GAB.rearrange("p a c -> p (a c)"),
                        )
                        nc.vector.tensor_copy(gtap[:, 2 * CW :], TC[:])
                        GTAP[0] = gtap

            # ---- U phase: et contributions for all batches at once ----
            # token t = p*16+k  ->  U column for (p, k) is U[:, p*16+k];
            # the host pre-permutes U so tile k's columns are contiguous.
            for s in range(2):
                for k in range(KT):
                    nc.tensor.matmul(
                        psE[:, s, k, :],
                        U12s[:, s, k * 128 : (k + 1) * 128],
                        C12[:, s, :],
                        start=True,
                        stop=True,
                    )

            # ---- per-pair: logits, max (via PE transpose), exp, readout --
            NBIAS_TAPS = []
            for P in range(NP):
                xt = XB[P]
                et = wpool.tile([128, 2, 2, KT], F32, tag="et", bufs=2)
                nc.vector.scalar_tensor_tensor(
                    out=et[:],
                    in0=psE[:, :, :, 2 * P : 2 * P + 2].rearrange(
                        "p s k b -> p b s k"
                    ),
                    scalar=1.0,
                    in1=XWB[:, 2 * P : 2 * P + 2],
                    op0=ALU.mult,
                    op1=ALU.add,
                )
                # per-partition maxima, cols (bb, s)
                mxp = wpool.tile([128, 4], F32, tag="mxp", bufs=2)
                nc.vector.tensor_reduce(
                    out=mxp[:], in_=et[:], axis=mybir.AxisListType.X,
                    op=ALU.max,
                )
                # cross-partition max: PE transpose -> X-reduce -> negate
                # -> PE transpose -> K=1 ones-matmul broadcast
                psT = psX[0:4, 64 + 128 * P : 64 + 128 * (P + 1)]
                nc.tensor.transpose(psT, mxp[:], IDN[:])
                nmx = wpool.tile([4, 1], F32, tag="nmx", bufs=2)
                nc.vector.tensor_reduce(
                    out=nmx[:], in_=psT, axis=mybir.AxisListType.X, op=ALU.max
                )
                nmneg = wpool.tile([4, 1], F32, tag="nmneg", bufs=2)
                nc.vector.tensor_scalar_mul(nmneg[:], nmx[:], -1.0)
                psT2 = psX[0:1, 320 + 8 * P : 324 + 8 * P]
                nc.tensor.transpose(psT2, nmneg[:], IDN[0:4, 0:4])
                nmrow = wpool.tile([1, 4], F32, tag="nmrow", bufs=2)
                nc.vector.tensor_copy(nmrow[:], psT2)
                psM = psX[:, 336 + 16 * P : 340 + 16 * P]
                nc.tensor.matmul(
                    psM, ONES[:], nmrow[:], start=True, stop=True
                )
                nbias = wpool.tile([128, 4], F32, tag="nbias", bufs=2)
                nc.vector.tensor_copy(nbias[:], psM)
                if DEBUG_TAPS:
                    tap = cpool.tile([128, 8], F32, tag=f"tap{P}")
                    nc.vector.tensor_copy(tap[:, 0:4], nbias[:])
                    nc.vector.tensor_copy(tap[:, 4:8], mxp[:])
                    NBIAS_TAPS.append(tap)

                EX = wpool.tile([128, 2, 2, KT], F16, tag="ex", bufs=2)
                for bb in range(2):
                    for s in range(2):
                        nc.scalar.activation(
                            EX[:, bb, s, :], et[:, bb, s, :], AF.Exp,
                            bias=nbias[:, 2 * bb + s : 2 * bb + s + 1],
                        )

                # readout: 4 concurrent col-group streams; slot j = 2*s+bb
                # -> PE col-group j, PSUM partition 32*j.  Z lands at col E
                # via the ones column.  One PSUM bank per slot: accumulation
                # groups must not share a bank (bank-wide has_written clear).
                psOs = []
                for j in range(4):
                    psO_j = pspool.tile([128, CW], F32, tag=f"psO{j}", bufs=1)
                    psOs.append(psO_j)
                for k in range(KT):
                    for bb in range(2):
                        for s in range(2):
                            j = 2 * s + bb
                            nc.tensor.matmul(
                                psOs[j][32 * j : 32 * j + 1, :],
                                EX[:, bb, s, k : k + 1],
                                xt[:, bb, s, k, :],
                                start=(k == 0),
                                stop=(k == KT - 1),
                                tile_position=(0, 32 * j),
                            )
                # normalize: out = o~ / Z
                rz = wpool.tile([128, 1], F32, tag="rz", bufs=2)
                for bb in range(2):
                    b = 2 * P + bb
                    for s in range(2):
                        j = 2 * s + bb
                        p0 = 32 * j
                        nc.vector.reciprocal(
                            rz[p0 : p0 + 1, :],
                            psOs[j][p0 : p0 + 1, E : E + 1],
                        )
                        nc.scalar.mul(
                            OUT[p0 : p0 + 1, b * E : (b + 1) * E],
                            psOs[j][p0 : p0 + 1, 0:E],
                            rz[p0 : p0 + 1, :],
                        )

            if DEBUG_TAPS:
                DBG = cpool.tile([128, 152], F32, tag="dbg")
                nc.vector.tensor_copy(
                    DBG[:, 0:8], C12.rearrange("p s b -> p (s b)")
                )
                nc.vector.tensor_copy(
                    DBG[:, 8:136], XWB.rearrange("p b s k -> p (b s k)")
                )
                nc.vector.tensor_copy(DBG[:, 136:144], NBIAS_TAPS[0])
                nc.vector.tensor_copy(DBG[:, 144:152], NBIAS_TAPS[1])
                nc.sync.dma_start(dbgd, DBG[:])
                nc.sync.dma_start(gdbgd, GTAP[0][:])

            # out rows: side s batches (bb, bb+2) live on partition 32*(2s+bb)
            for s in range(2):
                for bb in range(2):
                    p0 = 32 * (2 * s + bb)
                    src = OUT[p0 : p0 + 1, :].rearrange(
                        "p (P bb e) -> p bb P e", bb=2, e=E
                    )
                    dst = outd[s].rearrange("(P bb e) -> bb P e", bb=2, e=E)
                    nc.sync.dma_start(
                        dst[bb].unsqueeze(0), src[:, bb]
                    )

    return nc


_NC_CACHE = {}


def _get_nc():
    if "nc" not in _NC_CACHE:
        _NC_CACHE["nc"] = _build()
    return _NC_CACHE["nc"]


# U column permutation: tile k, lane j  <-  U[:, j*16 + k]
_UIDX = np.arange(T).reshape(128, KT).T.reshape(-1)


def _prep_in_maps(x1, x2, W1, b1, U1, W2, b2, U2):
    f16 = np.float16
    x1 = np.asarray(x1, dtype=np.float32)
    x2 = np.asarray(x2, dtype=np.float32)

    # packed x: (B, 2, 128, KT, CW) fp16, token t = p*16 + k, ones col at E
    xall = np.zeros((B, 2, 128, KT, CW), dtype=f16)
    xall[:, 0, :, :, 0:E] = x1.reshape(B, 128, KT, E).astype(f16)
    xall[:, 1, :, :, 0:E] = x2.reshape(B, 128, KT, E).astype(f16)
    xall[:, :, :, :, E] = 1.0

    u12 = np.stack(
        [
            np.asarray(U1, np.float32)[:, _UIDX].astype(f16),
            np.asarray(U2, np.float32)[:, _UIDX].astype(f16),
        ]
    )
    # j = 2*bb + s -> side s = j % 2
    w12 = np.ascontiguousarray(
        np.broadcast_to(
            np.stack(
                [np.asarray(W, f16)[:, 0] for W in (W1, W2, W1, W2)]
            )[None, :, :],
            (128, 4, E),
        )
    )
    b12 = np.ascontiguousarray(
        np.stack(
            [
                np.asarray(bv, f16)[:, 0].reshape(128, KT)
                for bv in (b1, b2, b1, b2)
            ],
            axis=1,
        )
    )
    ident = np.eye(128, dtype=np.float32)

    in_maps = []
    for c in range(NCORES):
        sl = slice(c * BPC, (c + 1) * BPC)
        in_maps.append(
            {
                "xc": np.ascontiguousarray(xall[sl]).reshape(
                    BPC * 2, 128, KT, CW
                ),
                "u12": u12,
                "w12bc": w12,
                "b12s": b12,
                "ident": ident,
            }
        )
    return in_maps


def _run(trace=False, tmpdir=None, **inputs):
    nc = _get_nc()
    if not _NC_CACHE.get("legalized"):
        # must happen after any CoreSim use (sim can't model bare wait-nops)
        _legalize_sync_waits(nc)
        _NC_CACHE["legalized"] = True
    in_maps = _prep_in_maps(**inputs)
    res = run_bass_kernel_spmd(
        nc, in_maps, list(range(NCORES)), trace=trace, tmpdir=tmpdir
    )
    # per-core out: (2, BPC*E) -> (BPC, 2E)
    outs = []
    for r in res.results:
        o = r["out"].reshape(2, BPC, E)
        outs.append(np.concatenate([o[0], o[1]], axis=1))
    out = np.concatenate(outs, axis=0)
    return out, res


def kernel(x1, x2, W1, b1, U1, W2, b2, U2):
    out, _ = _run(
        x1=x1, x2=x2, W1=W1, b1=b1, U1=U1, W2=W2, b2=b2, U2=U2
    )
    return out


# revision 44
# speedup vs baseline: 1.2058x; 1.1429x over previous
"""AttentionMM kernel for Trainium2 (Bass/Tile), data-parallel over 8 NeuronCores.

Math (per batch b, with x1,x2: (T,E)):
    S = x1 @ x2^T  is never materialized:
        t1 = sum_i x1[i,:] ;  t2 = sum_j x2[j,:]
        G2 = x1^T @ x2  (E,E)
        c1 = (1/T) G2^T t2 ;  c2 = (1/T) G2 t1   (via G = x2^T x1 = G2^T)
    et1 = c1 @ U1 + x1 @ W1 + b1 ;  et2 = c2 @ U2 + x2 @ W2 + b2
    o1 = softmax(et1) @ x1 ;  o2 = softmax(et2) @ x2 ;  out = [o1 | o2]

Implementation notes:
  - Everything on-chip is float16: fp16 matmuls run at 1 cycle/row on the PE
    (vs 4 cycles/row for f32r below 256 moving columns) and halve HBM
    traffic.  PSUM accumulation stays fp32; measured rel err ~4e-3.
  - Tokens sit in SBUF partitions, p-major: token t = p*16 + k.  Batches are
    packed in PAIRS ([128, 2, 2, KT, CW] tiles) so elementwise stages run as
    one DVE op per pair.
  - The host appends a ones-column to x: Gram matmuls yield t1/t2 for free
    and readout matmuls yield the softmax denominator Z for free.
  - Softmax subtracts a true per-(batch,side) max so exp() stays in fp16
    range for any input scale.  The cross-partition max/broadcast runs on
    the PE (transpose-mode with a host-supplied fp16 identity + a K=1
    ones-matmul broadcast) - the walrus here can't encode gpsimd partition
    reduces, and DVE stream-shuffle tables inflate the NEFF preamble.
  - x@W runs on DVE: fp16 multiply at 2x rate, then a tensor_tensor fold
    tree (DVE's plain reduce runs at 1x, tensor_tensor at 2x).
  - Readout packs (side, batch%2) into the PE's four 32-wide column groups
    via tile_position, so four N=130 streams run concurrently and each
    (batch, side) readout row lands on its own partition (0/32/64/96),
    which also keeps the per-side normalize ops on legal partition bases.
"""

import numpy as np

import concourse.bass as bass
import concourse.mybir as mybir
import concourse.tile as tile
from concourse.bass_utils import run_bass_kernel_spmd

B, T, E = 32, 2048, 128
NCORES = 8
BPC = B // NCORES            # batches per core
NP = BPC // 2                # batch pairs per core
KT = T // 128                # token tiles per batch
CW = E + 2                   # row width: 128 x-cols + ones col + pad
F32 = mybir.dt.float32
F16 = mybir.dt.float16
AF = mybir.ActivationFunctionType
ALU = mybir.AluOpType
INV_T = 1.0 / T
DEBUG_TAPS = False


def _patch_sem_clear():
    """The installed walrus cannot encode EVENT_SEMAPHORE_RANGE_CLEAR (raw
    ISA, "ISA wrong length"), which TileContext's exit path emits via
    gpsimd.sem_clear. Skip the clear (keep the DMA drain + bookkeeping);
    the runtime re-initializes semaphore state per NEFF execution."""
    if getattr(bass.Bass, "_semclear_patched", False):
        return
    from concourse.bass import compact_to_ranges

    def patched(self, sems):
        if not sems:
            return
        sem_nums = [s.num if hasattr(s, "num") else s for s in sems]
        for sem_range in compact_to_ranges(sem_nums):
            assert self._state.free_isdisjoint(sem_range)
            self.gpsimd.dma_reset(sem_range)
        self._state.prepend_free_semaphores(sem_nums)
        for poison_set in self._tile_sem_poison_stack:
            poison_set.update(sem_nums)

    bass.Bass.clear_and_free_semaphores = patched
    bass.Bass._semclear_patched = True


def _legalize_sync_waits(nc):
    """The installed walrus encodes at most one sync-wait per instruction
    ("Too many sync wait commands"). Move excess waits onto engine NoOps
    inserted immediately before the instruction — same engine, same
    program position, so semantics are unchanged."""
    import bass_rust

    fn = nc.m.functions[0]
    n_nops = 0
    for blk in fn.blocks:
        insts = blk.instructions
        out = []
        dirty = False
        for inst in insts:
            si = inst.sync_info
            if si is not None and len(si.on_wait) > 1:
                waits = list(si.on_wait)
                for w in waits[:-1]:
                    nop = mybir.InstNoOp(
                        name=f"waitnop-{n_nops}", engine=inst.engine
                    )
                    nop.sync_info = bass_rust.SyncInfo(
                        on_wait=[w], on_update=[]
                    )
                    out.append(nop)
                    n_nops += 1
                inst.sync_info = bass_rust.SyncInfo(
                    on_wait=[waits[-1]], on_update=list(si.on_update)
                )
                dirty = True
            out.append(inst)
        if dirty:
            blk.instructions = out
    return n_nops


def _build():
    _patch_sem_clear()
    nc = bass.Bass(
        "TRN2", target_bir_lowering=False, debug=False, num_devices=NCORES
    )

    # x: (pair, bb, side) packed, p-major tokens: row = x_s[2P+bb, p*16+k]
    xd = nc.dram_tensor(
        "xc", (BPC * 2, 128, KT, CW), F16, kind="ExternalInput"
    ).ap()
    ud = nc.dram_tensor("u12", (2, E, T), F16, kind="ExternalInput").ap()
    wd = nc.dram_tensor("w12bc", (128, 4, E), F16, kind="ExternalInput").ap()
    bd = nc.dram_tensor("b12s", (128, 4, KT), F16, kind="ExternalInput").ap()
    idd = nc.dram_tensor("ident", (128, 128), F32, kind="ExternalInput").ap()
    outd = nc.dram_tensor("out", (2, BPC * E), F32, kind="ExternalOutput").ap()
    if DEBUG_TAPS:
        dbgd = nc.dram_tensor(
            "dbg", (128, 152), F32, kind="ExternalOutput"
        ).ap()
        gdbgd = nc.dram_tensor(
            "gdbg", (128, 2 * CW + 2), F32, kind="ExternalOutput"
        ).ap()

    with tile.TileContext(nc) as tc:
        with (
            tc.tile_pool(name="const", bufs=1) as cpool,
            tc.tile_pool(name="xpool", bufs=1) as xpool,
            tc.tile_pool(name="work", bufs=2) as wpool,
            tc.tile_pool(name="ps", bufs=1, space="PSUM") as pspool,
        ):
            # ---- persistent tiles ----
            U12s = cpool.tile([128, 2, T], F16, tag="u12")
            # W/b pre-expanded on host to 4 cols (j = 2*bb + s) so the
            # pair-merged DVE ops keep affine 3-free-dim APs
            W12 = cpool.tile([128, 4, E], F16, tag="w12")
            B12 = cpool.tile([128, 4, KT], F16, tag="b12")
            IDN = cpool.tile([128, 128], F32, tag="ident")
            ONES = cpool.tile([1, 128], F32, tag="ones")
            C12 = cpool.tile([128, 2, BPC], F16, tag="c12")
            XWB = cpool.tile([128, BPC, 2, KT], F16, tag="xwb")
            OUT = cpool.tile([128, BPC * E], F32, tag="outbuf")
            nc.vector.memset(ONES[:], 1.0)

            # Params + identity + U on the ACT ring first; ALL x1 on the
            # sync ring, x2 on the ACT ring behind the params.
            nc.scalar.dma_start(W12[:], wd)
            nc.scalar.dma_start(B12[:], bd)

            XB = []
            for P in range(NP):
                xt = xpool.tile([128, 2, 2, KT, CW], F16, tag=f"x_{P}")
                XB.append(xt)
            KH = KT // 2
            for P in range(NP):
                xt = XB[P]
                scr = wpool.tile([128, 2, 2, KT, E], F16, tag="scr", bufs=1)
                for bb in range(2):
                    b = 2 * P + bb
                    if b == 0:
                        for h in range(2):
                            ks = slice(h * KH, (h + 1) * KH)
                            nc.sync.dma_start(
                                xt[:, bb, 0, ks, :], xd[2 * b][:, ks]
                            )
                            nc.scalar.dma_start(
                                xt[:, bb, 1, ks, :], xd[2 * b + 1][:, ks]
                            )
                    else:
                        nc.sync.dma_start(xt[:, bb, 0], xd[2 * b])
                        nc.scalar.dma_start(xt[:, bb, 1], xd[2 * b + 1])
                    # x@W multiply per batch (starts as soon as this batch
                    # lands); folds run pair-wide below.
                    nc.vector.tensor_tensor(
                        scr[:, bb],
                        xt[:, bb, :, :, 0:E],
                        W12[:, 2 * bb : 2 * bb + 2, :].unsqueeze(2)
                        .broadcast_to((128, 2, KT, E)),
                        ALU.mult,
                    )
                src = scr.rearrange("p b s k c -> p (b s) k c")
                w = E
                while w > 1:
                    h = w // 2
                    dst = wpool.tile(
                        [128, 4, KT, h], F16, tag=f"xf{h}", bufs=1
                    )
                    nc.vector.tensor_tensor(
                        dst[:], src[:, :, :, 0:h], src[:, :, :, h:w],
                        ALU.add,
                    )
                    src = dst
                    w = h
                nc.vector.tensor_add(
                    XWB[:, 2 * P : 2 * P + 2].rearrange(
                        "p b s k -> p (b s) k"
                    ),
                    src[:, :, :, 0],
                    B12[:],
                )

            nc.sync.dma_start(U12s[:, 0, :], ud[0])
            nc.sync.dma_start(U12s[:, 1, :], ud[1])
            # identity only needed at the max stage (~late); last on ACT ring
            nc.scalar.dma_start(IDN[:], idd)

            psE = pspool.tile([128, 2, KT, BPC], F32, tag="psE", bufs=1)
            # one shared bank for the small PSUM pieces of the max stage
            # (slices are disjoint; writes/reads are naturally ordered);
            # a second fp16-typed bank for PE-transpose outputs
            psX = pspool.tile([128, 512], F32, tag="psX", bufs=1)

            # ---- per-batch: Gram phases, c's ----
            GTAP = {}
            for P in range(NP):
                for bb in range(2):
                    b = 2 * P + bb
                    xt = XB[P]

                    # [G2 | t1] = x1^T @ [x2 | 1] ; [G | t2] = x2^T @ [x1 | 1]
                    # NOTE: start=True clears has_written BANK-wide, so the
                    # two accumulation groups sharing this bank must run
                    # sequentially (A fully, then B) — finished values
                    # persist, only the accumulate flags are cleared.
                    psAB = pspool.tile([128, 2, CW], F32, tag="psAB", bufs=2)
                    for k in range(KT):
                        nc.tensor.matmul(
                            psAB[:, 0, :],
                            xt[:, bb, 0, k, 0:E],
                            xt[:, bb, 1, k, :],
                            start=(k == 0),
                            stop=(k == KT - 1),
                        )
                    for k in range(KT):
                        nc.tensor.matmul(
                            psAB[:, 1, :],
                            xt[:, bb, 1, k, 0:E],
                            xt[:, bb, 0, k, :],
                            start=(k == 0),
                            stop=(k == KT - 1),
                        )

                    GAB = wpool.tile([128, 2, CW], F16, tag="gab", bufs=2)
                    nc.scalar.copy(GAB[:], psAB[:])

                    # TC = [t1 | t2]; then:
                    #   lhsT=G2, rhs=TC -> col1: G2^T t2 = T*c1
                    #   lhsT=G,  rhs=TC -> col0: G^T t1  = T*c2
                    TC = wpool.tile([128, 2], F16, tag="tc", bufs=2)
                    nc.scalar.copy(TC[:], GAB[:, :, E])
                    psC = psX[:, 8 * b : 8 * b + 4]
                    nc.tensor.matmul(
                        psC[:, 0:2], GAB[:, 0, 0:E], TC[:],
                        start=True, stop=True,
                    )
                    nc.tensor.matmul(
                        psC[:, 2:4], GAB[:, 1, 0:E], TC[:],
                        start=True, stop=True,
                    )
                    # scale by 1/T while casting into the batched c matrix
                    nc.scalar.mul(C12[:, 0, b : b + 1], psC[:, 1:2], INV_T)
                    nc.scalar.mul(C12[:, 1, b : b + 1], psC[:, 2:3], INV_T)
                    if DEBUG_TAPS and b == 0:
                        gtap = cpool.tile([128, 2 * CW + 2], F32, tag="gtap")
                        nc.vector.tensor_copy(
                            gtap[:, 0 : 2 * CW],
                            GAB.rearrange("p a c -> p (a c)"),
                        )
                        nc.vector.tensor_copy(gtap[:, 2 * CW :], TC[:])
                        GTAP[0] = gtap

            # ---- U phase: et contributions for all batches at once ----
            # token t = p*16+k  ->  U column for (p, k) is U[:, p*16+k];
            # the host pre-permutes U so tile k's columns are contiguous.
            for s in range(2):
                for k in range(KT):
                    nc.tensor.matmul(
                        psE[:, s, k, :],
                        U12s[:, s, k * 128 : (k + 1) * 128],
                        C12[:, s, :],
                        start=True,
                        stop=True,
                    )

            # ---- phase A: logits, cross-partition max, exp (both pairs) --
            NBIAS_TAPS = []
            EXs = []
            for P in range(NP):
                et = wpool.tile([128, 2, 2, KT], F32, tag="et", bufs=2)
                nc.vector.scalar_tensor_tensor(
                    out=et[:],
                    in0=psE[:, :, :, 2 * P : 2 * P + 2].rearrange(
                        "p s k b -> p b s k"
                    ),
                    scalar=1.0,
                    in1=XWB[:, 2 * P : 2 * P + 2],
                    op0=ALU.mult,
                    op1=ALU.add,
                )
                # per-partition maxima, cols (bb, s)
                mxp = wpool.tile([128, 4], F32, tag="mxp", bufs=2)
                nc.vector.tensor_reduce(
                    out=mxp[:], in_=et[:], axis=mybir.AxisListType.X,
                    op=ALU.max,
                )
                # cross-partition max: PE transpose -> X-reduce -> negate
                # -> PE transpose -> K=1 ones-matmul broadcast
                psT = psX[0:4, 64 + 128 * P : 64 + 128 * (P + 1)]
                nc.tensor.transpose(psT, mxp[:], IDN[:])
                nmx = wpool.tile([4, 1], F32, tag="nmx", bufs=2)
                nc.vector.tensor_reduce(
                    out=nmx[:], in_=psT, axis=mybir.AxisListType.X, op=ALU.max
                )
                nmneg = wpool.tile([4, 1], F32, tag="nmneg", bufs=2)
                nc.vector.tensor_scalar_mul(nmneg[:], nmx[:], -1.0)
                psT2 = psX[0:1, 320 + 8 * P : 324 + 8 * P]
                nc.tensor.transpose(psT2, nmneg[:], IDN[0:4, 0:4])
                nmrow = wpool.tile([1, 4], F32, tag="nmrow", bufs=2)
                nc.vector.tensor_copy(nmrow[:], psT2)
                psM = psX[:, 336 + 16 * P : 340 + 16 * P]
                nc.tensor.matmul(
                    psM, ONES[:], nmrow[:], start=True, stop=True
                )
                nbias = wpool.tile([128, 4], F32, tag="nbias", bufs=2)
                nc.vector.tensor_copy(nbias[:], psM)
                if DEBUG_TAPS:
                    tap = cpool.tile([128, 8], F32, tag=f"tap{P}")
                    nc.vector.tensor_copy(tap[:, 0:4], nbias[:])
                    nc.vector.tensor_copy(tap[:, 4:8], mxp[:])
                    NBIAS_TAPS.append(tap)

                EX = wpool.tile([128, 2, 2, KT], F16, tag="ex", bufs=2)
                for bb in range(2):
                    for s in range(2):
                        nc.scalar.activation(
                            EX[:, bb, s, :], et[:, bb, s, :], AF.Exp,
                            bias=nbias[:, 2 * bb + s : 2 * bb + s + 1],
                        )
                EXs.append(EX)

            # ---- phase B: readout + normalize + store (both pairs) ----
            # slot j = 2*s+bb -> PE col-group j, PSUM partition 32*j; four
            # concurrent N=130 streams.  One PSUM bank per slot (bank-wide
            # has_written clear forbids sharing between accumulation groups).
            psOs = []
            for j in range(4):
                psO_j = pspool.tile([128, CW], F32, tag=f"psO{j}", bufs=1)
                psOs.append(psO_j)
            for P in range(NP):
                xt = XB[P]
                EX = EXs[P]
                for k in range(KT):
                    for bb in range(2):
                        for s in range(2):
                            j = 2 * s + bb
                            nc.tensor.matmul(
                                psOs[j][32 * j : 32 * j + 1, :],
                                EX[:, bb, s, k : k + 1],
                                xt[:, bb, s, k, :],
                                start=(k == 0),
                                stop=(k == KT - 1),
                                tile_position=(0, 32 * j),
                            )
                # normalize: out = o~ / Z ; OUT row p0 holds pair-col P*E
                rz = wpool.tile([128, 1], F32, tag="rz", bufs=2)
                for bb in range(2):
                    for s in range(2):
                        j = 2 * s + bb
                        p0 = 32 * j
                        nc.vector.reciprocal(
                            rz[p0 : p0 + 1, :],
                            psOs[j][p0 : p0 + 1, E : E + 1],
                        )
                        nc.scalar.mul(
                            OUT[p0 : p0 + 1, P * E : (P + 1) * E],
                            psOs[j][p0 : p0 + 1, 0:E],
                            rz[p0 : p0 + 1, :],
                        )
                # store this pair (overlaps the other pair's readout):
                # side s rows {64s, 64s+32} (bb), cols P*E:(P+1)*E
                for s in range(2):
                    srcv = OUT.rearrange("(a r) n -> a r n", r=32)[
                        2 * s : 2 * s + 2, 0, P * E : (P + 1) * E
                    ]
                    dstv = outd[s].rearrange(
                        "(Pd bb e) -> Pd bb e", bb=2, e=E
                    )[P]
                    nc.sync.dma_start(dstv, srcv)

            if DEBUG_TAPS:
                DBG = cpool.tile([128, 152], F32, tag="dbg")
                nc.vector.tensor_copy(
                    DBG[:, 0:8], C12.rearrange("p s b -> p (s b)")
                )
                nc.vector.tensor_copy(
                    DBG[:, 8:136], XWB.rearrange("p b s k -> p (b s k)")
                )
                nc.vector.tensor_copy(DBG[:, 136:144], NBIAS_TAPS[0])
                nc.vector.tensor_copy(DBG[:, 144:152], NBIAS_TAPS[1])
                nc.sync.dma_start(dbgd, DBG[:])
                nc.sync.dma_start(gdbgd, GTAP[0][:])

    return nc


_NC_CACHE = {}


def _get_nc():
    if "nc" not in _NC_CACHE:
        _NC_CACHE["nc"] = _build()
    return _NC_CACHE["nc"]


# U column permutation: tile k, lane j  <-  U[:, j*16 + k]
_UIDX = np.arange(T).reshape(128, KT).T.reshape(-1)


def _prep_in_maps(x1, x2, W1, b1, U1, W2, b2, U2):
    f16 = np.float16
    x1 = np.asarray(x1, dtype=np.float32)
    x2 = np.asarray(x2, dtype=np.float32)

    # packed x: (B, 2, 128, KT, CW) fp16, token t = p*16 + k, ones col at E
    xall = np.zeros((B, 2, 128, KT, CW), dtype=f16)
    xall[:, 0, :, :, 0:E] = x1.reshape(B, 128, KT, E).astype(f16)
    xall[:, 1, :, :, 0:E] = x2.reshape(B, 128, KT, E).astype(f16)
    xall[:, :, :, :, E] = 1.0

    u12 = np.stack(
        [
            np.asarray(U1, np.float32)[:, _UIDX].astype(f16),
            np.asarray(U2, np.float32)[:, _UIDX].astype(f16),
        ]
    )
    # j = 2*bb + s -> side s = j % 2
    w12 = np.ascontiguousarray(
        np.broadcast_to(
            np.stack(
                [np.asarray(W, f16)[:, 0] for W in (W1, W2, W1, W2)]
            )[None, :, :],
            (128, 4, E),
        )
    )
    b12 = np.ascontiguousarray(
        np.stack(
            [
                np.asarray(bv, f16)[:, 0].reshape(128, KT)
                for bv in (b1, b2, b1, b2)
            ],
            axis=1,
        )
    )
    ident = np.eye(128, dtype=np.float32)

    in_maps = []
    for c in range(NCORES):
        sl = slice(c * BPC, (c + 1) * BPC)
        in_maps.append(
            {
                "xc": np.ascontiguousarray(xall[sl]).reshape(
                    BPC * 2, 128, KT, CW
                ),
                "u12": u12,
                "w12bc": w12,
                "b12s": b12,
                "ident": ident,
            }
        )
    return in_maps


def _run(trace=False, tmpdir=None, **inputs):
    nc = _get_nc()
    if not _NC_CACHE.get("legalized"):
        # must happen after any CoreSim use (sim can't model bare wait-nops)
        _legalize_sync_waits(nc)
        _NC_CACHE["legalized"] = True
    in_maps = _prep_in_maps(**inputs)
    res = run_bass_kernel_spmd(
        nc, in_maps, list(range(NCORES)), trace=trace, tmpdir=tmpdir
    )
    # per-core out: (2, BPC*E) -> (BPC, 2E)
    outs = []
    for r in res.results:
        o = r["out"].reshape(2, BPC, E)
        outs.append(np.concatenate([o[0], o[1]], axis=1))
    out = np.concatenate(outs, axis=0)
    return out, res


def kernel(x1, x2, W1, b1, U1, W2, b2, U2):
    out, _ = _run(
        x1=x1, x2=x2, W1=W1, b1=b1, U1=U1, W2=W2, b2=b2, U2=U2
    )
    return out
